# revision 15
# baseline (speedup 1.0000x reference)
import numpy as np
from contextlib import ExitStack

B, T, D = 2, 2048, 1024
NH, NKV, HD, RD = 16, 4, 64, 32
NCORES = 8
HPG = NH // NKV
G = NKV
TK = 128
TQ = 512
NTK = T // TK
NTQ = T // TQ
DCH = D // 128
EM = NH * HD
QKV_M = HPG * HD + 2 * HD

PERM = list(range(0, 16)) + list(range(32, 48)) + list(range(16, 32)) + list(range(48, 64))

NEG = -1.0e30

_CACHE = {}


def _round_fp32r(a):
    u = np.ascontiguousarray(a, np.float32).view(np.uint32)
    u = ((u.astype(np.uint64) + 0x800) & 0xFFFFF000).astype(np.uint32)
    return u.view(np.float32)


def _build_program():
    import concourse.bass as bass
    import concourse.bacc as bacc
    import concourse.tile as tile
    import concourse.mybir as mybir
    from concourse.masks import make_identity

    F32 = mybir.dt.float32
    F32R = mybir.dt.float32r
    AT = mybir.AluOpType
    ACTF = mybir.ActivationFunctionType

    nc = bacc.Bacc("TRN2", target_bir_lowering=False, debug=False)

    xt_d = nc.dram_tensor("xt", [D, T], F32R, kind="ExternalInput").ap()
    wqkv_d = nc.dram_tensor("wqkv", [D, QKV_M], F32R, kind="ExternalInput").ap()
    wproj_d = nc.dram_tensor("wproj", [HPG * HD, D], F32R, kind="ExternalInput").ap()
    v0_d = nc.dram_tensor("v0s", [HPG, T, HD], F32, kind="ExternalInput").ap()
    oma_d = nc.dram_tensor("oma", [128, HPG], F32, kind="ExternalInput").ap()
    cos_d = nc.dram_tensor("cosT", [RD // 2, T], F32, kind="ExternalInput").ap()
    sin_d = nc.dram_tensor("sinT", [RD // 2, T], F32, kind="ExternalInput").ap()
    tri_d = nc.dram_tensor("tri", [TK, TK], F32R, kind="ExternalInput").ap()
    ones_d = nc.dram_tensor("onesd", [128, 64], F32R, kind="ExternalInput").ap()
    out_d = nc.dram_tensor("out", [T, D], F32, kind="ExternalOutput").ap()

    with tile.TileContext(nc) as tc, ExitStack() as ctx, \
            nc.allow_low_precision(reason="fp32r matmul operands (tf32-like) are intentional"):
        const = ctx.enter_context(tc.tile_pool(name="const", bufs=1))
        wq_pool = ctx.enter_context(tc.tile_pool(name="wq", bufs=1))
        xt_pool = ctx.enter_context(tc.tile_pool(name="xt", bufs=10))
        qkv_pool = ctx.enter_context(tc.tile_pool(name="qkv", bufs=1))
        vaug_pool = ctx.enter_context(tc.tile_pool(name="vaug", bufs=1))
        v0t_pool = ctx.enter_context(tc.tile_pool(name="v0t", bufs=16))
        exp_pool = ctx.enter_context(tc.tile_pool(name="exp", bufs=3))
        nrm_pool = ctx.enter_context(tc.tile_pool(name="nrm", bufs=2))
        ynorm_pool = ctx.enter_context(tc.tile_pool(name="ynorm", bufs=1))
        out_pool = ctx.enter_context(tc.tile_pool(name="outs", bufs=3))
        ps1 = ctx.enter_context(tc.tile_pool(name="ps1", bufs=2, space="PSUM"))
        psy = ctx.enter_context(tc.tile_pool(name="psy", bufs=1, space="PSUM"))

        cos4 = const.tile([128, T], F32)
        sin4 = const.tile([128, T], F32)
        for bb in (0, 32, 64, 96):
            nc.scalar.dma_start(cos4[bb:bb + 16, :], cos_d)
            nc.scalar.dma_start(sin4[bb:bb + 16, :], sin_d)
        tri_sb = const.tile([TK, TK], F32R)
        nc.scalar.dma_start(tri_sb[:], tri_d)
        ident64 = const.tile([64, 64], F32)
        make_identity(nc, ident64[:])
        ones64 = const.tile([1, 64], F32R)
        nc.scalar.dma_start(ones64[:], ones_d[0:1, :])
        oma_bc = const.tile([128, HPG], F32)
        nc.scalar.dma_start(oma_bc[:], oma_d)

        w_qkv = wq_pool.tile([128, DCH, QKV_M], F32R)
        for dc in range(DCH):
            nc.sync.dma_start(
                w_qkv[:, dc, :], wqkv_d[128 * dc:128 * (dc + 1), :])

        qraw01 = qkv_pool.tile([128, T], F32R)
        qraw23 = qkv_pool.tile([128, T], F32R)
        k_sb = qkv_pool.tile([64, T], F32R)
        v_sb = qkv_pool.tile([64, T], F32)
        k2 = qkv_pool.tile([128, T], F32R)
        for jj in range(NTQ):
            xts = []
            for dc in range(DCH):
                xt = xt_pool.tile([128, TQ], F32R, tag="xt")
                nc.gpsimd.dma_start(
                    xt[:], xt_d[128 * dc:128 * (dc + 1), TQ * jj:TQ * (jj + 1)])
                xts.append(xt)
            for m in (2, 0, 1):
                ps = ps1.tile([128, TQ], F32, tag="ps1")
                for dc in range(DCH):
                    nc.tensor.matmul(
                        ps[:], w_qkv[:, dc, 128 * m:128 * (m + 1)], xts[dc][:],
                        start=(dc == 0), stop=(dc == DCH - 1))
                if m < 2:
                    dest = qraw01 if m == 0 else qraw23
                    nc.vector.tensor_copy(dest[:, TQ * jj:TQ * (jj + 1)], ps[:])
                else:
                    nc.vector.tensor_copy(k_sb[:, TQ * jj:TQ * (jj + 1)], ps[0:64, :])
                    nc.vector.tensor_copy(v_sb[:, TQ * jj:TQ * (jj + 1)], ps[64:128, :])

        def rope(tile_, base):
            b1, b2 = base, base + 32
            for hf in range(2):
                cl = slice(1024 * hf, 1024 * (hf + 1))
                x1 = tile_[b1:b1 + 16, cl]
                x2 = tile_[b2:b2 + 16, cl]
                tt = ps1.tile([48, 1024], F32, tag="ps1")
                t1 = tt[0:16, :]
                t2 = tt[32:48, :]
                nc.vector.tensor_tensor(t1, x1, sin4[b1:b1 + 16, cl], op=AT.mult)
                nc.vector.tensor_tensor(x1, x1, cos4[b1:b1 + 16, cl], op=AT.mult)
                nc.vector.tensor_tensor(t2, x2, sin4[b2:b2 + 16, cl], op=AT.mult)
                nc.vector.tensor_tensor(x2, x2, cos4[b2:b2 + 16, cl], op=AT.mult)
                nc.vector.tensor_tensor(x1, x1, t2, op=AT.subtract)
                nc.vector.tensor_tensor(x2, x2, t1, op=AT.add)

        rope(k_sb, 0)
        nc.sync.dma_start(k2[64:128, :], k_sb[:])

        rope(qraw01, 0)

        v0_part = [[None] * 4 for _ in range(HPG)]
        for q4 in range(4):
            for h in range(HPG):
                v0t = v0t_pool.tile([128, 4, HD], F32, tag="v0t")
                nc.gpsimd.dma_start(
                    v0t[:],
                    v0_d[h, 512 * q4:512 * (q4 + 1), :].rearrange(
                        "(c p) d -> p c d", p=128))
                v0_part[h][q4] = v0t
        v_aug = []
        for h in range(HPG):
            va = vaug_pool.tile([128, NTK, HD + 1], F32R, tag=f"va_{h}")
            nc.scalar.dma_start(va[:, :, HD:HD + 1], ones_d[:, 0:NTK])
            v_aug.append(va)
        for c in range(NTK):
            vt_ps = ps1.tile([128, HD], F32, tag="ps1")
            nc.tensor.transpose(vt_ps[:], v_sb[:, TK * c:TK * (c + 1)], ident64[:])
            for h in range(HPG):
                nc.vector.scalar_tensor_tensor(
                    out=v_aug[h][:, c, 0:HD], in0=vt_ps[:], scalar=oma_bc[:, h:h + 1],
                    in1=v0_part[h][c // 4][:, c % 4, :], op0=AT.mult, op1=AT.add)

        ynorm01 = ynorm_pool.tile([128, T], F32R)
        ynorm23 = ynorm_pool.tile([128, T], F32R)

        def head(h, extra=None):
            qtile = qraw01 if h < 2 else qraw23
            qb = 64 * (h % 2)
            ktile = k_sb if h % 2 == 0 else k2
            kb = 64 * (h % 2)
            y_ps = psy.tile([65, T], F32, tag="y")

            def scores(j):
                tqs = TK * j
                et = exp_pool.tile([128, T], F32R, tag="exp")
                for w in range(tqs // 1024, 2):
                    ws = max(tqs - 1024 * w, 0)
                    sc = ps1.tile([128, 1024], F32, tag="ps1")
                    for half in range(2):
                        s0 = max(ws, TQ * half)
                        ce = TQ * (half + 1)
                        if s0 >= ce:
                            continue
                        nc.tensor.matmul(
                            sc[:, s0:ce],
                            ktile[kb:kb + 64, TK * j:TK * (j + 1)],
                            qtile[qb:qb + 64, 1024 * w + s0:1024 * w + ce],
                            start=True, stop=True)
                    nc.scalar.activation(
                        et[:, 1024 * w + ws:1024 * (w + 1)], sc[:, ws:1024], ACTF.Exp)
                nc.gpsimd.tensor_tensor(
                    et[:, tqs:tqs + TK], et[:, tqs:tqs + TK], tri_sb[:], op=AT.mult)
                return et

            def pv(j, et):
                tqs = TK * j
                for jj in range(tqs // TQ, NTQ):
                    cs0 = max(TQ * jj, tqs)
                    nc.tensor.matmul(
                        y_ps[:, cs0:TQ * (jj + 1)],
                        v_aug[h][:, j, :],
                        et[:, cs0:TQ * (jj + 1)],
                        start=(j == 0), stop=(j == 4 * jj + 3))

            ets = scores(0)
            for j in range(NTK):
                if extra is not None:
                    extra(j)
                nxt = scores(j + 1) if j + 1 < NTK else None
                pv(j, ets)
                ets = nxt
            ydest = ynorm01 if h < 2 else ynorm23
            yb = 64 * (h % 2)
            rec = nrm_pool.tile([1, T], F32R, tag="rec")
            nc.vector.reciprocal(rec[:], y_ps[64:65, :])
            for jj in range(NTQ):
                bc_ps = ps1.tile([64, TQ], F32, tag="ps1")
                nc.tensor.matmul(bc_ps[:], ones64[:],
                                 rec[0:1, TQ * jj:TQ * (jj + 1)], start=True, stop=True)
                bc_sb = nrm_pool.tile([128, TQ], F32, tag="bc")
                nc.vector.tensor_copy(bc_sb[yb:yb + 64, :], bc_ps[:])
                nc.vector.tensor_tensor(
                    ydest[yb:yb + 64, TQ * jj:TQ * (jj + 1)],
                    y_ps[0:64, TQ * jj:TQ * (jj + 1)], bc_sb[yb:yb + 64, :], op=AT.mult)

        head(0, extra=lambda j: rope(qraw01, 64) if j == 4 else None)
        head(1, extra=lambda j: (rope(qraw23, 0) if j == 4 else
                                 (rope(qraw23, 64) if j == 9 else None)))

        w_proj = const.tile([128, 2, D], F32R)
        nc.sync.dma_start(w_proj[:], wproj_d.rearrange("(c p) e -> p c e", p=128))

        head(2)
        head(3)

        for t16 in range(NTK):
            for ec in range(2):
                op_ps = ps1.tile([128, TQ], F32, tag="ps1")
                nc.tensor.matmul(
                    op_ps[:], ynorm01[:, TK * t16:TK * (t16 + 1)],
                    w_proj[:, 0, TQ * ec:TQ * (ec + 1)], start=True, stop=False)
                nc.tensor.matmul(
                    op_ps[:], ynorm23[:, TK * t16:TK * (t16 + 1)],
                    w_proj[:, 1, TQ * ec:TQ * (ec + 1)], start=False, stop=True)
                ob = out_pool.tile([128, TQ], F32, tag="ob")
                nc.scalar.copy(ob[:], op_ps[:])
                nc.gpsimd.dma_start(
                    out_d[TK * t16:TK * (t16 + 1), TQ * ec:TQ * (ec + 1)], ob[:])

    nc.compile()
    return nc


def get_program():
    if "nc" not in _CACHE:
        _CACHE["nc"] = _build_program()
    return _CACHE["nc"]


def make_in_maps(x, cos, sin, v0, Wqkv, Wproj, vrl_alpha, qk_scale):
    x = np.asarray(x, np.float32)
    cos = np.asarray(cos, np.float32)
    sin = np.asarray(sin, np.float32)
    v0 = np.asarray(v0, np.float32)
    Wqkv = np.asarray(Wqkv, np.float32)
    Wproj = np.asarray(Wproj, np.float32)
    vrl_alpha = np.asarray(vrl_alpha, np.float32)
    qk_scale = np.asarray(qk_scale, np.float32)

    alpha = 1.0 / (1.0 + np.exp(-vrl_alpha.astype(np.float64)))
    alpha = alpha.astype(np.float32)
    perm = np.asarray(PERM)

    cosT = np.ascontiguousarray(cos.reshape(T, RD // 2).T).astype(np.float32)
    sinT = np.ascontiguousarray(sin.reshape(T, RD // 2).T).astype(np.float32)
    rr = np.arange(TK)
    tri = np.where(rr[None, :] >= rr[:, None], 1.0, 0.0).astype(np.float32)

    xts = [_round_fp32r(np.ascontiguousarray(x[b].T)) for b in range(B)]

    in_maps = []
    for c in range(NCORES):
        b, g = divmod(c, G)
        heads = [HPG * g + i for i in range(HPG)]
        wq = Wqkv[256 * g:256 * (g + 1)].reshape(HPG, HD, D)[:, perm, :]
        wq = wq * (qk_scale[heads].astype(np.float64) / HD).astype(np.float32)[:, None, None]
        wk = Wqkv[EM + HD * g:EM + HD * (g + 1)][perm]
        wv = Wqkv[EM + NKV * HD + HD * g:EM + NKV * HD + HD * (g + 1)]
        wl = np.concatenate([wq.reshape(HPG * HD, D), wk, wv], axis=0)
        wqkvT = _round_fp32r(np.ascontiguousarray(wl.T))
        wprojT = _round_fp32r(np.ascontiguousarray(Wproj[:, 256 * g:256 * (g + 1)].T))
        v0s = np.ascontiguousarray(alpha[heads][:, None, None] * v0[b, heads])
        oma = np.ascontiguousarray(
            np.broadcast_to((1.0 - alpha[heads]).reshape(1, HPG), (128, HPG)))
        in_maps.append({
            "xt": xts[b], "wqkv": wqkvT, "wproj": wprojT, "v0s": v0s,
            "oma": oma, "cosT": cosT, "sinT": sinT, "tri": tri,
            "onesd": np.ones((128, 64), np.float32),
        })
    return in_maps


def assemble(results):
    out = np.empty((B, T, D), np.float32)
    for b in range(B):
        acc = results[G * b]["out"].astype(np.float32)
        for g in range(1, G):
            acc = acc + results[G * b + g]["out"]
        out[b] = acc
    return out


def kernel(x, cos, sin, v0, Wqkv, Wproj, vrl_alpha, qk_scale):
    from concourse.bass_utils import run_bass_kernel_spmd

    nc = get_program()
    in_maps = make_in_maps(x, cos, sin, v0, Wqkv, Wproj, vrl_alpha, qk_scale)
    res = run_bass_kernel_spmd(nc, in_maps, core_ids=list(range(NCORES)))
    out = assemble(res.results)
    return out, np.asarray(v0, np.float32)


# revision 25
# speedup vs baseline: 1.0019x; 1.0019x over previous
import numpy as np
from contextlib import ExitStack

B, T, D = 2, 2048, 1024
NH, NKV, HD, RD = 16, 4, 64, 32
NCORES = 8
HPG = NH // NKV
G = NKV
TK = 128
TQ = 512
NTK = T // TK
NTQ = T // TQ
DCH = D // 128
EM = NH * HD
QKV_M = HPG * HD + 2 * HD

PERM = list(range(0, 16)) + list(range(32, 48)) + list(range(16, 32)) + list(range(48, 64))

NEG = -1.0e30

_CACHE = {}


def _round_fp32r(a):
    u = np.ascontiguousarray(a, np.float32).view(np.uint32)
    u = ((u.astype(np.uint64) + 0x800) & 0xFFFFF000).astype(np.uint32)
    return u.view(np.float32)


def _build_program():
    import concourse.bass as bass
    import concourse.bacc as bacc
    import concourse.tile as tile
    import concourse.mybir as mybir
    from concourse.masks import make_identity

    F32 = mybir.dt.float32
    F32R = mybir.dt.float32r
    AT = mybir.AluOpType
    ACTF = mybir.ActivationFunctionType

    nc = bacc.Bacc("TRN2", target_bir_lowering=False, debug=False)

    xt_d = nc.dram_tensor("xt", [D, T], F32R, kind="ExternalInput").ap()
    wqkv_d = nc.dram_tensor("wqkv", [D, QKV_M], F32R, kind="ExternalInput").ap()
    wproj_d = nc.dram_tensor("wproj", [HPG * HD, D], F32R, kind="ExternalInput").ap()
    v0_d = nc.dram_tensor("v0s", [HPG, T, HD], F32, kind="ExternalInput").ap()
    oma_d = nc.dram_tensor("oma", [128, HPG], F32, kind="ExternalInput").ap()
    cos_d = nc.dram_tensor("cosT", [RD // 2, T], F32, kind="ExternalInput").ap()
    sin_d = nc.dram_tensor("sinT", [RD // 2, T], F32, kind="ExternalInput").ap()
    tri_d = nc.dram_tensor("tri", [TK, TK], F32R, kind="ExternalInput").ap()
    ones_d = nc.dram_tensor("onesd", [128, 64], F32R, kind="ExternalInput").ap()
    out_d = nc.dram_tensor("out", [T, D], F32, kind="ExternalOutput").ap()

    with tile.TileContext(nc) as tc, ExitStack() as ctx, \
            nc.allow_low_precision(reason="fp32r matmul operands (tf32-like) are intentional"):
        const = ctx.enter_context(tc.tile_pool(name="const", bufs=1))
        wq_pool = ctx.enter_context(tc.tile_pool(name="wq", bufs=1))
        xt_pool = ctx.enter_context(tc.tile_pool(name="xt", bufs=10))
        qkv_pool = ctx.enter_context(tc.tile_pool(name="qkv", bufs=1))
        vaug_pool = ctx.enter_context(tc.tile_pool(name="vaug", bufs=1))
        v0t_pool = ctx.enter_context(tc.tile_pool(name="v0t", bufs=16))
        exp_pool = ctx.enter_context(tc.tile_pool(name="exp", bufs=3))
        nrm_pool = ctx.enter_context(tc.tile_pool(name="nrm", bufs=2))
        ynorm_pool = ctx.enter_context(tc.tile_pool(name="ynorm", bufs=1))
        out_pool = ctx.enter_context(tc.tile_pool(name="outs", bufs=3))
        ps1 = ctx.enter_context(tc.tile_pool(name="ps1", bufs=2, space="PSUM"))
        psy = ctx.enter_context(tc.tile_pool(name="psy", bufs=1, space="PSUM"))

        cos4 = const.tile([128, T], F32)
        sin4 = const.tile([128, T], F32)
        for bb in (0, 32, 64, 96):
            nc.scalar.dma_start(cos4[bb:bb + 16, :], cos_d)
            nc.scalar.dma_start(sin4[bb:bb + 16, :], sin_d)
        tri_sb = const.tile([TK, TK], F32R)
        nc.scalar.dma_start(tri_sb[:], tri_d)
        ident64 = const.tile([64, 64], F32)
        make_identity(nc, ident64[:])
        ones64 = const.tile([1, 64], F32R)
        nc.scalar.dma_start(ones64[:], ones_d[0:1, :])
        oma_bc = const.tile([128, HPG], F32)
        nc.scalar.dma_start(oma_bc[:], oma_d)

        w_qkv = wq_pool.tile([128, DCH, QKV_M], F32R)
        for dc in range(DCH):
            nc.sync.dma_start(
                w_qkv[:, dc, :], wqkv_d[128 * dc:128 * (dc + 1), :])

        qraw01 = qkv_pool.tile([128, T], F32R)
        qraw23 = qkv_pool.tile([128, T], F32R)
        k_sb = qkv_pool.tile([64, T], F32R)
        v_sb = qkv_pool.tile([64, T], F32)
        k2 = qkv_pool.tile([128, T], F32R)
        for jj in range(NTQ):
            xts = []
            for dc in range(DCH):
                xt = xt_pool.tile([128, TQ], F32R, tag="xt")
                eng = nc.gpsimd if dc % 2 == 0 else nc.sync
                eng.dma_start(
                    xt[:], xt_d[128 * dc:128 * (dc + 1), TQ * jj:TQ * (jj + 1)])
                xts.append(xt)
            for m in (2, 0, 1):
                ps = ps1.tile([128, TQ], F32, tag="ps1")
                for dc in range(DCH):
                    nc.tensor.matmul(
                        ps[:], w_qkv[:, dc, 128 * m:128 * (m + 1)], xts[dc][:],
                        start=(dc == 0), stop=(dc == DCH - 1))
                if m < 2:
                    dest = qraw01 if m == 0 else qraw23
                    nc.vector.tensor_copy(dest[:, TQ * jj:TQ * (jj + 1)], ps[:])
                else:
                    nc.vector.tensor_copy(k_sb[:, TQ * jj:TQ * (jj + 1)], ps[0:64, :])
                    nc.vector.tensor_copy(v_sb[:, TQ * jj:TQ * (jj + 1)], ps[64:128, :])

        def rope(tile_, base):
            b1, b2 = base, base + 32
            for hf in range(2):
                cl = slice(1024 * hf, 1024 * (hf + 1))
                x1 = tile_[b1:b1 + 16, cl]
                x2 = tile_[b2:b2 + 16, cl]
                tt = ps1.tile([48, 1024], F32, tag="ps1")
                t1 = tt[0:16, :]
                t2 = tt[32:48, :]
                nc.vector.tensor_tensor(t1, x1, sin4[b1:b1 + 16, cl], op=AT.mult)
                nc.vector.tensor_tensor(x1, x1, cos4[b1:b1 + 16, cl], op=AT.mult)
                nc.vector.tensor_tensor(t2, x2, sin4[b2:b2 + 16, cl], op=AT.mult)
                nc.vector.tensor_tensor(x2, x2, cos4[b2:b2 + 16, cl], op=AT.mult)
                nc.vector.tensor_tensor(x1, x1, t2, op=AT.subtract)
                nc.vector.tensor_tensor(x2, x2, t1, op=AT.add)

        rope(k_sb, 0)
        nc.sync.dma_start(k2[64:128, :], k_sb[:])

        rope(qraw01, 0)

        v0_part = [[None] * 4 for _ in range(HPG)]
        for q4 in range(4):
            for h in range(HPG):
                v0t = v0t_pool.tile([128, 4, HD], F32, tag="v0t")
                nc.gpsimd.dma_start(
                    v0t[:],
                    v0_d[h, 512 * q4:512 * (q4 + 1), :].rearrange(
                        "(c p) d -> p c d", p=128))
                v0_part[h][q4] = v0t
        v_aug = []
        for h in range(HPG):
            va = vaug_pool.tile([128, NTK, HD + 1], F32R, tag=f"va_{h}")
            nc.scalar.dma_start(va[:, :, HD:HD + 1], ones_d[:, 0:NTK])
            v_aug.append(va)
        for c in range(NTK):
            vt_ps = ps1.tile([128, HD], F32, tag="ps1")
            nc.tensor.transpose(vt_ps[:], v_sb[:, TK * c:TK * (c + 1)], ident64[:])
            for h in range(HPG):
                nc.vector.scalar_tensor_tensor(
                    out=v_aug[h][:, c, 0:HD], in0=vt_ps[:], scalar=oma_bc[:, h:h + 1],
                    in1=v0_part[h][c // 4][:, c % 4, :], op0=AT.mult, op1=AT.add)

        ynorm01 = ynorm_pool.tile([128, T], F32R)
        ynorm23 = ynorm_pool.tile([128, T], F32R)

        def head(h, extra=None):
            qtile = qraw01 if h < 2 else qraw23
            qb = 64 * (h % 2)
            ktile = k_sb if h % 2 == 0 else k2
            kb = 64 * (h % 2)
            y_ps = psy.tile([65, T], F32, tag="y")

            def scores(j):
                tqs = TK * j
                et = exp_pool.tile([128, T], F32R, tag="exp")
                for w in range(tqs // 1024, 2):
                    ws = max(tqs - 1024 * w, 0)
                    sc = ps1.tile([128, 1024], F32, tag="ps1")
                    for half in range(2):
                        s0 = max(ws, TQ * half)
                        ce = TQ * (half + 1)
                        if s0 >= ce:
                            continue
                        nc.tensor.matmul(
                            sc[:, s0:ce],
                            ktile[kb:kb + 64, TK * j:TK * (j + 1)],
                            qtile[qb:qb + 64, 1024 * w + s0:1024 * w + ce],
                            start=True, stop=True)
                    nc.scalar.activation(
                        et[:, 1024 * w + ws:1024 * (w + 1)], sc[:, ws:1024], ACTF.Exp)
                nc.gpsimd.tensor_tensor(
                    et[:, tqs:tqs + TK], et[:, tqs:tqs + TK], tri_sb[:], op=AT.mult)
                return et

            def pv(j, et):
                tqs = TK * j
                for jj in range(tqs // TQ, NTQ):
                    cs0 = max(TQ * jj, tqs)
                    nc.tensor.matmul(
                        y_ps[:, cs0:TQ * (jj + 1)],
                        v_aug[h][:, j, :],
                        et[:, cs0:TQ * (jj + 1)],
                        start=(j == 0), stop=(j == 4 * jj + 3))

            ets = scores(0)
            for j in range(NTK):
                if extra is not None:
                    extra(j)
                nxt = scores(j + 1) if j + 1 < NTK else None
                pv(j, ets)
                ets = nxt
            ydest = ynorm01 if h < 2 else ynorm23
            yb = 64 * (h % 2)
            rec = nrm_pool.tile([1, T], F32R, tag="rec")
            nc.vector.reciprocal(rec[:], y_ps[64:65, :])
            for jj in range(NTQ):
                bc_ps = ps1.tile([64, TQ], F32, tag="ps1")
                nc.tensor.matmul(bc_ps[:], ones64[:],
                                 rec[0:1, TQ * jj:TQ * (jj + 1)], start=True, stop=True)
                bc_sb = nrm_pool.tile([128, TQ], F32, tag="bc")
                nc.vector.tensor_copy(bc_sb[yb:yb + 64, :], bc_ps[:])
                nc.vector.tensor_tensor(
                    ydest[yb:yb + 64, TQ * jj:TQ * (jj + 1)],
                    y_ps[0:64, TQ * jj:TQ * (jj + 1)], bc_sb[yb:yb + 64, :], op=AT.mult)

        head(0, extra=lambda j: rope(qraw01, 64) if j == 4 else None)
        head(1, extra=lambda j: (rope(qraw23, 0) if j == 4 else
                                 (rope(qraw23, 64) if j == 9 else None)))

        w_proj = const.tile([128, 2, D], F32R)
        nc.sync.dma_start(w_proj[:], wproj_d.rearrange("(c p) e -> p c e", p=128))

        head(2)
        head(3)

        for t16 in range(NTK):
            for ec in range(2):
                op_ps = ps1.tile([128, TQ], F32, tag="ps1")
                nc.tensor.matmul(
                    op_ps[:], ynorm01[:, TK * t16:TK * (t16 + 1)],
                    w_proj[:, 0, TQ * ec:TQ * (ec + 1)], start=True, stop=False)
                nc.tensor.matmul(
                    op_ps[:], ynorm23[:, TK * t16:TK * (t16 + 1)],
                    w_proj[:, 1, TQ * ec:TQ * (ec + 1)], start=False, stop=True)
                ob = out_pool.tile([128, TQ], F32, tag="ob")
                nc.scalar.copy(ob[:], op_ps[:])
                eng = nc.gpsimd if (t16 + ec) % 2 == 0 else nc.sync
                eng.dma_start(
                    out_d[TK * t16:TK * (t16 + 1), TQ * ec:TQ * (ec + 1)], ob[:])

    nc.compile()
    return nc


def get_program():
    if "nc" not in _CACHE:
        _CACHE["nc"] = _build_program()
    return _CACHE["nc"]


def make_in_maps(x, cos, sin, v0, Wqkv, Wproj, vrl_alpha, qk_scale):
    x = np.asarray(x, np.float32)
    cos = np.asarray(cos, np.float32)
    sin = np.asarray(sin, np.float32)
    v0 = np.asarray(v0, np.float32)
    Wqkv = np.asarray(Wqkv, np.float32)
    Wproj = np.asarray(Wproj, np.float32)
    vrl_alpha = np.asarray(vrl_alpha, np.float32)
    qk_scale = np.asarray(qk_scale, np.float32)

    alpha = 1.0 / (1.0 + np.exp(-vrl_alpha.astype(np.float64)))
    alpha = alpha.astype(np.float32)
    perm = np.asarray(PERM)

    cosT = np.ascontiguousarray(cos.reshape(T, RD // 2).T).astype(np.float32)
    sinT = np.ascontiguousarray(sin.reshape(T, RD // 2).T).astype(np.float32)
    rr = np.arange(TK)
    tri = np.where(rr[None, :] >= rr[:, None], 1.0, 0.0).astype(np.float32)

    xts = [_round_fp32r(np.ascontiguousarray(x[b].T)) for b in range(B)]

    in_maps = []
    for c in range(NCORES):
        b, g = divmod(c, G)
        heads = [HPG * g + i for i in range(HPG)]
        wq = Wqkv[256 * g:256 * (g + 1)].reshape(HPG, HD, D)[:, perm, :]
        wq = wq * (qk_scale[heads].astype(np.float64) / HD).astype(np.float32)[:, None, None]
        wk = Wqkv[EM + HD * g:EM + HD * (g + 1)][perm]
        wv = Wqkv[EM + NKV * HD + HD * g:EM + NKV * HD + HD * (g + 1)]
        wl = np.concatenate([wq.reshape(HPG * HD, D), wk, wv], axis=0)
        wqkvT = _round_fp32r(np.ascontiguousarray(wl.T))
        wprojT = _round_fp32r(np.ascontiguousarray(Wproj[:, 256 * g:256 * (g + 1)].T))
        v0s = np.ascontiguousarray(alpha[heads][:, None, None] * v0[b, heads])
        oma = np.ascontiguousarray(
            np.broadcast_to((1.0 - alpha[heads]).reshape(1, HPG), (128, HPG)))
        in_maps.append({
            "xt": xts[b], "wqkv": wqkvT, "wproj": wprojT, "v0s": v0s,
            "oma": oma, "cosT": cosT, "sinT": sinT, "tri": tri,
            "onesd": np.ones((128, 64), np.float32),
        })
    return in_maps


def assemble(results):
    out = np.empty((B, T, D), np.float32)
    for b in range(B):
        acc = results[G * b]["out"].astype(np.float32)
        for g in range(1, G):
            acc = acc + results[G * b + g]["out"]
        out[b] = acc
    return out


def kernel(x, cos, sin, v0, Wqkv, Wproj, vrl_alpha, qk_scale):
    from concourse.bass_utils import run_bass_kernel_spmd

    nc = get_program()
    in_maps = make_in_maps(x, cos, sin, v0, Wqkv, Wproj, vrl_alpha, qk_scale)
    res = run_bass_kernel_spmd(nc, in_maps, core_ids=list(range(NCORES)))
    out = assemble(res.results)
    return out, np.asarray(v0, np.float32)
